# revision 9
# baseline (speedup 1.0000x reference)
"""Trainium2 Bass kernel for nn_EquiLinearRegToReg.

Math: out[b,p,j,y] = sum_{i,x} weights[i, j, (y-x)%K] * field_feat[b,p,i,x]
Shapes: field_feat [8, 512, 256, 16] f32, weights [256, 256, 16] f32
        -> out [8, 512, 256, 16] f32.

Strategy: data-parallel over batch (1 batch of M=512 rows per core).
Per core this is a [512, 4096] @ [4096, 4096] matmul where the right
operand is the block-circulant expansion of weights. The 16 circular
shifts are materialized on the host (32 MB in fp16) and STREAMED from
DRAM as contiguous slabs, so every matmul's moving operand is a fully
contiguous [128, 2*32*16] AP (a strided windowed AP costs +25ns/MM in
AP-walk overhead; contiguous hits the 216ns/MM pair floor at N=512).

Inputs are cast to fp16 on the host: fp32r matmuls self-load weights
(+107ns/MM serialized); fp16 gets a separate FWL LDWEIGHTS that the PE
pulls ahead of in-flight matmuls, so weight loads are free. PSUM
accumulation is fp32; fp16 mantissa (10 bits) keeps the result within
~3e-4 relative error.

Loop structure: 4 groups of 8 PSUM banks (2 out-column tiles x 4
row-chunks); each group accumulates over all 32 K-tiles, then evicts
PSUM->SBUF->DRAM while the next group computes.
"""

import os
import numpy as np

import concourse.bass as bass
import concourse.mybir as mybir
import concourse.tile as tile
from concourse import bacc
from concourse.bass_utils import run_bass_kernel_spmd

BATCH, NUM_PART, IN_FEAT, OUT_FEAT, K = 8, 512, 256, 256, 16
N_CORES = 8
P = 128
IO = IN_FEAT // P          # 2 partition-tiles over in_features
KT = K * IO                # 32 K-tiles of 128, kt = io*16 + x
BPC = NUM_PART // P        # 4 chunks of 128 rows
NT = OUT_FEAT * K // 512   # 8 output column tiles of 512
JPN = OUT_FEAT // NT       # 32 j's per output tile
# Column tiles per group: 8 PSUM banks = len(group)*BPC. The last two
# groups are single-tile so the final evictions overlap compute.
GROUPS = [[0, 1], [2, 3], [4, 5], [6], [7]]

_CACHE = {}


def _build():
    """Build + compile the per-core Bass program (cached)."""
    if "nc" in _CACHE:
        return _CACHE["nc"]

    f32 = mybir.dt.float32
    f16 = mybir.dt.float16

    nc = bacc.Bacc(None, target_bir_lowering=False, debug=False)
    # fieldT[kt, i128, bp] : K-major transposed field shard, kt = io*16 + x
    field_d = nc.dram_tensor("fieldT", [KT, P, NUM_PART], f16, kind="ExternalInput")
    # wd[nt, kt, i128, j32, y16] : pre-shifted weight slabs
    wd_d = nc.dram_tensor("wd", [NT, KT, P, JPN, K], f16, kind="ExternalInput")
    out_d = nc.dram_tensor("out", [NUM_PART, OUT_FEAT * K], f32, kind="ExternalOutput")

    with tile.TileContext(nc) as tc:
        with (
            tc.tile_pool(name="fpool", bufs=1) as fpool,
            tc.tile_pool(name="wpool", bufs=36) as wpool,
            tc.tile_pool(name="opool", bufs=8) as opool,
            tc.tile_pool(name="psum", bufs=8, space="PSUM") as psum,
        ):
            # PE warmup: ~40 dependency-free matmuls on a zeroed scratch
            # tile get the HAM clock gate to 8/8 during the DMA/preamble
            # head, so the first real matmuls run at 2.4 GHz.
            wu = fpool.tile([P, P], f16, name="wu", tag="wu", bufs=1)
            nc.vector.memset(wu[:], 0.0)
            wacc = psum.tile([P, P], f32, tag="ps", name="wacc")
            for _ in range(40):
                nc.tensor.matmul(wacc[:], wu[:], wu[:], start=True, stop=True)

            ft = fpool.tile([P, KT, NUM_PART], f16, tag="ft", bufs=1, name="ft")

            # Group-0 weight slabs interleaved with field slabs so the
            # kt-sweep can start immediately and stays ahead of DMA.
            ws0 = []
            for kt in range(KT):
                w = wpool.tile([P, len(GROUPS[0]), JPN, K], f16, tag="ws0",
                               bufs=32, name=f"ws0_{kt}")
                for li, nt in enumerate(GROUPS[0]):
                    nc.sync.dma_start(w[:, li], wd_d[nt, kt])
                ws0.append(w)
                nc.sync.dma_start(ft[:, kt, :], field_d[kt])

            for g, nts in enumerate(GROUPS):
                accs = [
                    psum.tile([P, 512], f32, tag="ps", name=f"ps_{g}_{i}")
                    for i in range(BPC * len(nts))
                ]
                for kt in range(KT):
                    if g == 0:
                        w = ws0[kt]
                    else:
                        w = wpool.tile([P, len(nts), JPN, K], f16, tag="ws",
                                       name=f"ws{g}_{kt}")
                        for li, nt in enumerate(nts):
                            nc.sync.dma_start(w[:, li], wd_d[nt, kt])
                    for bpc in range(BPC):
                        lhsT = ft[:, kt, bpc * P:(bpc + 1) * P]
                        for li in range(len(nts)):
                            nc.tensor.matmul(
                                accs[bpc * len(nts) + li][:],
                                lhsT,
                                w[:, li],
                                start=(kt == 0),
                                stop=(kt == KT - 1),
                            )
                for bpc in range(BPC):
                    for li, nt in enumerate(nts):
                        ot = opool.tile([P, 512], f32, tag="ot",
                                        name=f"ot_{g}_{bpc}_{li}")
                        nc.vector.tensor_copy(ot[:], accs[bpc * len(nts) + li][:])
                        nc.sync.dma_start(
                            out_d[bpc * P:(bpc + 1) * P, nt * 512:(nt + 1) * 512],
                            ot[:],
                        )

    nc.compile()
    _CACHE["nc"] = nc
    return nc


def _prep_inputs(field_feat: np.ndarray, weights: np.ndarray):
    field_feat = np.ascontiguousarray(field_feat, dtype=np.float32)
    weights = np.ascontiguousarray(weights, dtype=np.float32)

    # rolled[x, i, j, y] = weights[i, j, (y-x) % K]
    rolled = np.stack([np.roll(weights, x, axis=2) for x in range(K)])
    # wd[nt, io*K+x, i128, j, y] = rolled[x, io*128+i128, nt*JPN+j, y]
    wd = rolled.reshape(K, IO, P, NT, JPN, K).transpose(3, 1, 0, 2, 4, 5)
    wd = np.ascontiguousarray(wd.reshape(NT, KT, P, JPN, K), dtype=np.float16)

    in_maps = []
    for c in range(N_CORES):
        # fieldT[io*K+x, i128, bp]
        fT = field_feat[c].transpose(1, 2, 0)                  # [256i, 16x, 512bp]
        fT = fT.reshape(IO, P, K, NUM_PART).transpose(0, 2, 1, 3)
        fT = np.ascontiguousarray(fT.reshape(KT, P, NUM_PART), dtype=np.float16)
        in_maps.append({"fieldT": fT, "wd": wd})
    return in_maps


def kernel(field_feat: np.ndarray, weights: np.ndarray) -> np.ndarray:
    nc = _build()
    in_maps = _prep_inputs(field_feat, weights)
    trace = bool(int(os.environ.get("KERNEL_TRACE", "0")))
    res = run_bass_kernel_spmd(nc, in_maps, list(range(N_CORES)), trace=trace)
    if trace:
        kernel.last_exec_time_ns = res.exec_time_ns
        kernel.last_results = res
    out = np.stack([res.results[c]["out"] for c in range(N_CORES)], axis=0)
    return out.reshape(BATCH, NUM_PART, OUT_FEAT, K)


# revision 10
# speedup vs baseline: 1.0342x; 1.0342x over previous
"""Trainium2 Bass kernel for nn_EquiLinearRegToReg.

Math: out[b,p,j,y] = sum_{i,x} weights[i, j, (y-x)%K] * field_feat[b,p,i,x]
Shapes: field_feat [8, 512, 256, 16] f32, weights [256, 256, 16] f32
        -> out [8, 512, 256, 16] f32.

Strategy: data-parallel over batch (1 batch of M=512 rows per core).
Per core this is a [512, 4096] @ [4096, 4096] matmul where the right
operand is the block-circulant expansion of weights. The 16 circular
shifts are materialized on the host (32 MB in fp16) and STREAMED from
DRAM as contiguous slabs, so every matmul's moving operand is a fully
contiguous [128, 2*32*16] AP (a strided windowed AP costs +25ns/MM in
AP-walk overhead; contiguous hits the 216ns/MM pair floor at N=512).

Inputs are cast to fp16 on the host: fp32r matmuls self-load weights
(+107ns/MM serialized); fp16 gets a separate FWL LDWEIGHTS that the PE
pulls ahead of in-flight matmuls, so weight loads are free. PSUM
accumulation is fp32; fp16 mantissa (10 bits) keeps the result within
~3e-4 relative error.

Loop structure: 4 groups of 8 PSUM banks (2 out-column tiles x 4
row-chunks); each group accumulates over all 32 K-tiles, then evicts
PSUM->SBUF->DRAM while the next group computes.
"""

import os
import numpy as np

import concourse.bass as bass
import concourse.mybir as mybir
import concourse.tile as tile
from concourse import bacc
from concourse.bass_utils import run_bass_kernel_spmd

BATCH, NUM_PART, IN_FEAT, OUT_FEAT, K = 8, 512, 256, 256, 16
N_CORES = 8
P = 128
IO = IN_FEAT // P          # 2 partition-tiles over in_features
KT = K * IO                # 32 K-tiles of 128, kt = io*16 + x
BPC = NUM_PART // P        # 4 chunks of 128 rows
NT = OUT_FEAT * K // 512   # 8 output column tiles of 512
JPN = OUT_FEAT // NT       # 32 j's per output tile
# Column tiles per group: 8 PSUM banks = len(group)*BPC. The last two
# groups are single-tile so the final evictions overlap compute.
GROUPS = [[0, 1], [2, 3], [4, 5], [6], [7]]

_CACHE = {}


def _build():
    """Build + compile the per-core Bass program (cached)."""
    if "nc" in _CACHE:
        return _CACHE["nc"]

    f32 = mybir.dt.float32
    f16 = mybir.dt.float16

    nc = bacc.Bacc(None, target_bir_lowering=False, debug=False)
    # fieldT[kt, i128, bp] : K-major transposed field shard, kt = io*16 + x
    field_d = nc.dram_tensor("fieldT", [KT, P, NUM_PART], f16, kind="ExternalInput")
    # wd[kt, i128, nt, j32, y16] : pre-shifted weight slabs
    wd_d = nc.dram_tensor("wd", [KT, P, NT, JPN, K], f16, kind="ExternalInput")
    out_d = nc.dram_tensor("out", [NUM_PART, OUT_FEAT * K], f32, kind="ExternalOutput")

    with tile.TileContext(nc) as tc:
        with (
            tc.tile_pool(name="fpool", bufs=1) as fpool,
            tc.tile_pool(name="wpool", bufs=36) as wpool,
            tc.tile_pool(name="opool", bufs=8) as opool,
            tc.tile_pool(name="psum", bufs=8, space="PSUM") as psum,
        ):
            # PE warmup: ~40 dependency-free matmuls on a zeroed scratch
            # tile get the HAM clock gate to 8/8 during the DMA/preamble
            # head, so the first real matmuls run at 2.4 GHz.
            wu = fpool.tile([P, P], f16, name="wu", tag="wu", bufs=1)
            nc.vector.memset(wu[:], 0.0)
            wacc = psum.tile([P, P], f32, tag="ps", name="wacc")
            for _ in range(40):
                nc.tensor.matmul(wacc[:], wu[:], wu[:], start=True, stop=True)

            ft = fpool.tile([P, KT, NUM_PART], f16, tag="ft", bufs=1, name="ft")

            # Group-0 weight slabs interleaved with field slabs so the
            # kt-sweep can start immediately and stays ahead of DMA.
            ws0 = []
            for kt in range(KT):
                w = wpool.tile([P, len(GROUPS[0]), JPN, K], f16, tag="ws0",
                               bufs=32, name=f"ws0_{kt}")
                g0 = GROUPS[0]
                nc.sync.dma_start(w[:], wd_d[kt, :, g0[0]:g0[0] + len(g0)])
                ws0.append(w)
                nc.sync.dma_start(ft[:, kt, :], field_d[kt])

            for g, nts in enumerate(GROUPS):
                accs = [
                    psum.tile([P, 512], f32, tag="ps", name=f"ps_{g}_{i}")
                    for i in range(BPC * len(nts))
                ]
                for kt in range(KT):
                    if g == 0:
                        w = ws0[kt]
                    else:
                        w = wpool.tile([P, len(nts), JPN, K], f16, tag="ws",
                                       name=f"ws{g}_{kt}")
                        nc.sync.dma_start(
                            w[:], wd_d[kt, :, nts[0]:nts[0] + len(nts)])
                    for bpc in range(BPC):
                        lhsT = ft[:, kt, bpc * P:(bpc + 1) * P]
                        for li in range(len(nts)):
                            nc.tensor.matmul(
                                accs[bpc * len(nts) + li][:],
                                lhsT,
                                w[:, li],
                                start=(kt == 0),
                                stop=(kt == KT - 1),
                            )
                for bpc in range(BPC):
                    for li, nt in enumerate(nts):
                        ot = opool.tile([P, 512], f32, tag="ot",
                                        name=f"ot_{g}_{bpc}_{li}")
                        nc.vector.tensor_copy(ot[:], accs[bpc * len(nts) + li][:])
                        nc.sync.dma_start(
                            out_d[bpc * P:(bpc + 1) * P, nt * 512:(nt + 1) * 512],
                            ot[:],
                        )

    nc.compile()
    _CACHE["nc"] = nc
    return nc


def _prep_inputs(field_feat: np.ndarray, weights: np.ndarray):
    field_feat = np.ascontiguousarray(field_feat, dtype=np.float32)
    weights = np.ascontiguousarray(weights, dtype=np.float32)

    # rolled[x, i, j, y] = weights[i, j, (y-x) % K]
    rolled = np.stack([np.roll(weights, x, axis=2) for x in range(K)])
    # wd[io*K+x, i128, nt, j, y] = rolled[x, io*128+i128, nt*JPN+j, y]
    wd = rolled.reshape(K, IO, P, NT, JPN, K).transpose(1, 0, 2, 3, 4, 5)
    wd = np.ascontiguousarray(wd.reshape(KT, P, NT, JPN, K), dtype=np.float16)

    in_maps = []
    for c in range(N_CORES):
        # fieldT[io*K+x, i128, bp]
        fT = field_feat[c].transpose(1, 2, 0)                  # [256i, 16x, 512bp]
        fT = fT.reshape(IO, P, K, NUM_PART).transpose(0, 2, 1, 3)
        fT = np.ascontiguousarray(fT.reshape(KT, P, NUM_PART), dtype=np.float16)
        in_maps.append({"fieldT": fT, "wd": wd})
    return in_maps


def kernel(field_feat: np.ndarray, weights: np.ndarray) -> np.ndarray:
    nc = _build()
    in_maps = _prep_inputs(field_feat, weights)
    trace = bool(int(os.environ.get("KERNEL_TRACE", "0")))
    res = run_bass_kernel_spmd(nc, in_maps, list(range(N_CORES)), trace=trace)
    if trace:
        kernel.last_exec_time_ns = res.exec_time_ns
        kernel.last_results = res
    out = np.stack([res.results[c]["out"] for c in range(N_CORES)], axis=0)
    return out.reshape(BATCH, NUM_PART, OUT_FEAT, K)


# revision 11
# speedup vs baseline: 1.9132x; 1.8500x over previous
"""Trainium2 Bass kernel for nn_EquiLinearRegToReg.

Math: out[b,p,j,y] = sum_{i,x} weights[i, j, (y-x)%K] * field_feat[b,p,i,x]
Shapes: field_feat [8, 512, 256, 16] f32, weights [256, 256, 16] f32
        -> out [8, 512, 256, 16] f32.

Strategy: data-parallel over batch (1 batch of M=512 rows per core).
Per core this is a [512, 4096] @ [4096, 4096] matmul where the right
operand is the block-circulant expansion of weights. The 16 circular
shifts are materialized on the host (32 MB in fp16) and STREAMED from
DRAM as contiguous slabs, so every matmul's moving operand is a fully
contiguous [128, 2*32*16] AP (a strided windowed AP costs +25ns/MM in
AP-walk overhead; contiguous hits the 216ns/MM pair floor at N=512).

Inputs are cast to fp16 on the host: fp32r matmuls self-load weights
(+107ns/MM serialized); fp16 gets a separate FWL LDWEIGHTS that the PE
pulls ahead of in-flight matmuls, so weight loads are free. PSUM
accumulation is fp32; fp16 mantissa (10 bits) keeps the result within
~3e-4 relative error.

Loop structure: 4 groups of 8 PSUM banks (2 out-column tiles x 4
row-chunks); each group accumulates over all 32 K-tiles, then evicts
PSUM->SBUF->DRAM while the next group computes.
"""

import os
import numpy as np

import concourse.bass as bass
import concourse.mybir as mybir
import concourse.tile as tile
from concourse import bacc
from concourse.bass_utils import run_bass_kernel_spmd

BATCH, NUM_PART, IN_FEAT, OUT_FEAT, K = 8, 512, 256, 256, 16
N_CORES = 8
P = 128
IO = IN_FEAT // P          # 2 partition-tiles over in_features
KT = K * IO                # 32 K-tiles of 128, kt = io*16 + x
BPC = NUM_PART // P        # 4 chunks of 128 rows
NT = OUT_FEAT * K // 512   # 8 output column tiles of 512
JPN = OUT_FEAT // NT       # 32 j's per output tile
# Column tiles per group: 8 PSUM banks = len(group)*BPC. The last two
# groups are single-tile so the final evictions overlap compute.
GROUPS = [[0, 1], [2, 3], [4, 5], [6], [7]]

_CACHE = {}


def _build():
    """Build + compile the per-core Bass program (cached)."""
    if "nc" in _CACHE:
        return _CACHE["nc"]

    f32 = mybir.dt.float32
    f16 = mybir.dt.float16

    nc = bacc.Bacc(None, target_bir_lowering=False, debug=False)
    # fieldT[kt, i128, bp] : K-major transposed field shard, kt = io*16 + x
    field_d = nc.dram_tensor("fieldT", [KT, P, NUM_PART], f16, kind="ExternalInput")
    # wd[kt, i128, nt, j32, y16] : pre-shifted weight slabs
    wd_d = nc.dram_tensor("wd", [KT, P, NT, JPN, K], f16, kind="ExternalInput")
    out_d = nc.dram_tensor("out", [NUM_PART, OUT_FEAT * K], f32, kind="ExternalOutput")

    with tile.TileContext(nc) as tc:
        with (
            tc.tile_pool(name="fpool", bufs=1) as fpool,
            tc.tile_pool(name="wpool", bufs=36) as wpool,
            tc.tile_pool(name="opool", bufs=8) as opool,
            tc.tile_pool(name="psum", bufs=8, space="PSUM") as psum,
        ):
            # PE warmup: ~40 dependency-free matmuls on a zeroed scratch
            # tile get the HAM clock gate to 8/8 during the DMA/preamble
            # head, so the first real matmuls run at 2.4 GHz.
            wu = fpool.tile([P, P], f16, name="wu", tag="wu", bufs=1)
            nc.vector.memset(wu[:], 0.0)
            wacc = psum.tile([P, P], f32, tag="ps", name="wacc")
            for _ in range(40):
                nc.tensor.matmul(wacc[:], wu[:], wu[:], start=True, stop=True)

            ft = fpool.tile([P, KT, NUM_PART], f16, tag="ft", bufs=1, name="ft")

            # Group-0 weight slabs interleaved with field slabs so the
            # kt-sweep can start immediately and stays ahead of DMA.
            ws0 = []
            for kt in range(KT):
                w = wpool.tile([P, len(GROUPS[0]), JPN, K], f16, tag="ws0",
                               bufs=32, name=f"ws0_{kt}")
                g0 = GROUPS[0]
                nc.sync.dma_start(w[:], wd_d[kt, :, g0[0]:g0[0] + len(g0)])
                ws0.append(w)
                nc.sync.dma_start(ft[:, kt, :], field_d[kt])

            for g, nts in enumerate(GROUPS):
                accs = [
                    psum.tile([P, 512], f32, tag="ps", name=f"ps_{g}_{i}")
                    for i in range(BPC * len(nts))
                ]
                for kt in range(KT):
                    if g == 0:
                        w = ws0[kt]
                    else:
                        w = wpool.tile([P, len(nts), JPN, K], f16, tag="ws",
                                       name=f"ws{g}_{kt}")
                        nc.sync.dma_start(
                            w[:], wd_d[kt, :, nts[0]:nts[0] + len(nts)])
                    for bpc in range(BPC):
                        lhsT = ft[:, kt, bpc * P:(bpc + 1) * P]
                        for li in range(len(nts)):
                            nc.tensor.matmul(
                                accs[bpc * len(nts) + li][:],
                                lhsT,
                                w[:, li],
                                start=(kt == 0),
                                stop=(kt == KT - 1),
                            )
                for bpc in range(BPC):
                    for li, nt in enumerate(nts):
                        ot = opool.tile([P, 512], f32, tag="ot",
                                        name=f"ot_{g}_{bpc}_{li}")
                        nc.vector.tensor_copy(ot[:], accs[bpc * len(nts) + li][:])
                        nc.sync.dma_start(
                            out_d[bpc * P:(bpc + 1) * P, nt * 512:(nt + 1) * 512],
                            ot[:],
                        )

    nc.compile()
    _CACHE["nc"] = nc
    return nc


def _prep_inputs(field_feat: np.ndarray, weights: np.ndarray):
    field_feat = np.ascontiguousarray(field_feat, dtype=np.float32)
    weights = np.ascontiguousarray(weights, dtype=np.float32)

    # rolled[x, i, j, y] = weights[i, j, (y-x) % K]
    rolled = np.stack([np.roll(weights, x, axis=2) for x in range(K)])
    # wd[io*K+x, i128, nt, j, y] = rolled[x, io*128+i128, nt*JPN+j, y]
    wd = rolled.reshape(K, IO, P, NT, JPN, K).transpose(1, 0, 2, 3, 4, 5)
    wd = np.ascontiguousarray(wd.reshape(KT, P, NT, JPN, K), dtype=np.float16)

    in_maps = []
    for c in range(N_CORES):
        # fieldT[io*K+x, i128, bp]
        fT = field_feat[c].transpose(1, 2, 0)                  # [256i, 16x, 512bp]
        fT = fT.reshape(IO, P, K, NUM_PART).transpose(0, 2, 1, 3)
        fT = np.ascontiguousarray(fT.reshape(KT, P, NUM_PART), dtype=np.float16)
        in_maps.append({"fieldT": fT, "wd": wd})
    return in_maps


def kernel(field_feat: np.ndarray, weights: np.ndarray) -> np.ndarray:
    nc = _build()
    in_maps = _prep_inputs(field_feat, weights)
    trace = bool(int(os.environ.get("KERNEL_TRACE", "0")))
    # NRT occasionally reports a transient EXEC_UNIT_UNRECOVERABLE on the
    # first execute after a fresh session; a retry on a new session passes.
    last_exc = None
    for attempt in range(3):
        try:
            res = run_bass_kernel_spmd(nc, in_maps, list(range(N_CORES)),
                                       trace=trace)
            break
        except Exception as e:  # noqa: BLE001
            last_exc = e
            if attempt == 2:
                raise
    if trace:
        kernel.last_exec_time_ns = res.exec_time_ns
        kernel.last_results = res
    out = np.stack([res.results[c]["out"] for c in range(N_CORES)], axis=0)
    return out.reshape(BATCH, NUM_PART, OUT_FEAT, K)


# revision 12
# speedup vs baseline: 1.9873x; 1.0387x over previous
"""Device-pure Fourier-domain kernel for nn_EquiLinearRegToReg.

The operator is block-circulant over the k-axis: DFT diagonalization
reduces per-core MACs 8.6G -> ~1.1G. Three on-device stages:

S1: f_hat = DFT_x(field), computed as 32 matmuls with a block-diagonal
    DFT stationary (partition (x,i8) -> (i8,plane)), scattered to a
    DRAM scratch to re-layout partitions to (io,i127).
S2: per-frequency complex matmuls as stacked-K real matmuls
    (K = (re/im, io, i127), N = bp = 512), 120 matmuls.
S3: iDFT via block-diagonal stationary (partition (j8,plane) ->
    (j8,y)), 32 matmuls, evicted straight to the output.

Plane order: [w0, re1, im1, ..., re7, im7, w8] (16 real planes).
"""

import os
import numpy as np

import concourse.mybir as mybir
import concourse.tile as tile
from concourse import bacc
from concourse.bass_utils import run_bass_kernel_spmd

BATCH, NUM_PART, IN_FEAT, OUT_FEAT, K = 8, 512, 256, 256, 16
N_CORES = 8
P = 128
IO = IN_FEAT // P
NIG = IN_FEAT // 8          # 32 i-groups of 8
NJG = OUT_FEAT // 8         # 32 j-groups of 8
JC = OUT_FEAT // P          # 2 j-chunks of 128

_CACHE = {}

PLANES = [(0, "re")] + [(w, k) for w in range(1, 8) for k in ("re", "im")] + [(8, "re")]


def _cf():
    C = np.zeros((K, K))
    x = np.arange(K)
    for p, (w, kind) in enumerate(PLANES):
        C[:, p] = np.cos(2 * np.pi * w * x / K) if kind == "re" else -np.sin(2 * np.pi * w * x / K)
    return C


def _ci():
    C = np.zeros((K, K))
    y = np.arange(K)
    for p, (w, kind) in enumerate(PLANES):
        s = 1.0 / K if w in (0, 8) else 2.0 / K
        C[p, :] = s * np.cos(2 * np.pi * w * y / K) if kind == "re" else -s * np.sin(2 * np.pi * w * y / K)
    return C


def _s2_ktiles(pp):
    """For out-plane index pp: list of (contrib_plane_q, io) k-tiles.
    The matching stationary slabs live in W2[pp, kt]."""
    w, kind = PLANES[pp]
    if w in (0, 8):
        return [(pp, 0), (pp, 1)]
    if kind == "re":   # Hr = Fr Wr + Fi (-Wi)
        return [(pp, 0), (pp, 1), (pp + 1, 0), (pp + 1, 1)]
    else:              # Hi = Fr Wi + Fi Wr
        return [(pp - 1, 0), (pp - 1, 1), (pp, 0), (pp, 1)]


def _build():
    if "nc" in _CACHE:
        return _CACHE["nc"]
    f32 = mybir.dt.float32
    f16 = mybir.dt.float16

    nc = bacc.Bacc(None, target_bir_lowering=False, debug=False)
    fieldx_d = nc.dram_tensor("fieldx", [NIG // 4, P, 4, NUM_PART], f16, kind="ExternalInput")
    b1_d = nc.dram_tensor("b1", [P, P], f16, kind="ExternalInput")
    b3_d = nc.dram_tensor("b3", [P, P], f16, kind="ExternalInput")
    w2_d = nc.dram_tensor("w2", [K, P, 4, OUT_FEAT], f16, kind="ExternalInput")
    # scratch, laid out so every scatter/gather is one large affine DMA:
    # fh[i, p, bp] row-major: S1 writes [ig]-slabs, S2 reads p-strided rows
    fh_ds = [nc.dram_tensor(f"fh{h}", [NIG // 2, P, NUM_PART], f16) for h in range(2)]
    oh_ds = [nc.dram_tensor(f"oh{h}", [P, K, NUM_PART], f16) for h in range(2)]
    out_d = nc.dram_tensor("out", [NJG, P, NUM_PART], f32, kind="ExternalOutput")

    with tile.TileContext(nc) as tc:
        with (
            tc.tile_pool(name="const", bufs=1) as const,
            tc.tile_pool(name="sb", bufs=8) as sb,
            tc.tile_pool(name="st", bufs=8) as st,
            tc.tile_pool(name="psum", bufs=8, space="PSUM") as psum,
        ):
            # warmup
            wu = const.tile([P, P], f16, name="wu", tag="wu", bufs=1)
            nc.vector.memset(wu[:], 0.0)
            wacc = psum.tile([P, P], f32, tag="ps", name="wacc")
            for _ in range(40):
                nc.tensor.matmul(wacc[:], wu[:], wu[:], start=True, stop=True)

            b1 = const.tile([P, P], f16, name="b1", tag="b1", bufs=1)
            b3 = const.tile([P, P], f16, name="b3", tag="b3", bufs=1)
            w2t = const.tile([P, K, 4, OUT_FEAT], f16, name="w2t", tag="w2", bufs=1)
            fht = const.tile([P, IO, K, NUM_PART], f16, name="fht", tag="fh", bufs=1)
            nc.sync.dma_start(b1[:], b1_d[:])

            # ---- S1 ----  (fx batched 4 i-groups per DMA)
            fxs = []
            for b in range(NIG // 4):
                fx = sb.tile([P, 4, NUM_PART], f16, tag="fx", name=f"fx{b}")
                nc.sync.dma_start(fx[:], fieldx_d[b])
                fxs.append(fx)
            nc.sync.dma_start(b3[:], b3_d[:])
            for pp in range(K):
                nc.sync.dma_start(w2t[:, pp], w2_d[pp])
            fhvs = [
                fh_ds[h][:].rearrange("ig r bp -> (ig r) bp")
                .rearrange("(ig r) bp -> r ig bp", r=P)
                for h in range(2)
            ]
            for b in range(NIG // 4):
                sg = st.tile([P, 4, NUM_PART], f16, tag="sg", bufs=4,
                             name=f"sg{b}")
                for k4 in range(4):
                    ig = b * 4 + k4
                    acc = psum.tile([P, NUM_PART], f32, tag="ps", name=f"s1p{ig}")
                    nc.tensor.matmul(acc[:], b1[:], fxs[ig // 4][:, ig % 4, :],
                                     start=True, stop=True)
                    nc.vector.tensor_copy(sg[:, k4, :], acc[:])
                h, bh = divmod(b, NIG // 8)
                nc.scalar.dma_start(fhvs[h][:, bh * 4:(bh + 1) * 4, :], sg[:])

            # gather f_hat back in (io,i127)-partition layout; io half h
            # only depends on fh half h, so it overlaps S1's second half
            for io_ in range(IO):
                fh_flat = fh_ds[io_][:].rearrange("ig r bp -> (ig r) bp")
                fh_iq = fh_flat.rearrange("(i q) bp -> i q bp", q=K)
                nc.sync.dma_start(fht[:, io_], fh_iq)

            # ---- S2 ----  (jc outer so S3 can start after jc=0)
            for jc in range(JC):
                for pp in range(K):
                    kts = _s2_ktiles(pp)
                    acc = psum.tile([P, NUM_PART], f32, tag="ps", name=f"s2p{jc}_{pp}")
                    for ki, (q, io_) in enumerate(kts):
                        nc.tensor.matmul(
                            acc[:],
                            w2t[:, pp, ki, jc * P:(jc + 1) * P],
                            fht[:, io_, q, :],
                            start=(ki == 0),
                            stop=(ki == len(kts) - 1),
                        )
                    sg = st.tile([P, NUM_PART], f16, tag="hg", bufs=8, name=f"hg{jc}_{pp}")
                    nc.vector.tensor_copy(sg[:], acc[:])
                    nc.scalar.dma_start(oh_ds[jc][:, pp, :], sg[:])

            # ---- S3 ----
            ohvs = [
                oh_ds[jc][:].rearrange("j p bp -> (j p) bp")
                .rearrange("(jg r) bp -> r jg bp", r=P)
                for jc in range(JC)
            ]
            for bb in range(NJG // 8):
                jc, base = divmod(bb * 8, NJG // 2)
                oht = sb.tile([P, 8, NUM_PART], f16, tag="oht", bufs=3,
                              name=f"oht{bb}")
                nc.sync.dma_start(oht[:], ohvs[jc][:, base:base + 8, :])
                for j in range(8):
                    jg = bb * 8 + j
                    acc = psum.tile([P, NUM_PART], f32, tag="ps", name=f"s3p{jg}")
                    nc.tensor.matmul(acc[:], b3[:], oht[:, j, :],
                                     start=True, stop=True)
                    og = st.tile([P, NUM_PART], f32, tag="og", bufs=8,
                                 name=f"og{jg}")
                    nc.vector.tensor_copy(og[:], acc[:])
                    eng = nc.gpsimd if jg % 2 == 0 else nc.scalar
                    eng.dma_start(out_d[jg], og[:])

    nc.compile()
    _CACHE["nc"] = nc
    return nc


def _prep_inputs(field_feat, weights):
    field_feat = np.ascontiguousarray(field_feat, dtype=np.float32)
    weights = np.ascontiguousarray(weights, dtype=np.float32)

    Cf, Ci = _cf(), _ci()
    B1 = np.zeros((P, P), np.float32)
    for x in range(K):
        for i8 in range(8):
            B1[x * 8 + i8, i8 * 16:(i8 + 1) * 16] = Cf[x]
    B3 = np.zeros((P, P), np.float32)
    for j8 in range(8):
        B3[j8 * 16:(j8 + 1) * 16, j8 * 16:(j8 + 1) * 16] = Ci
    Wf = np.fft.fft(weights, axis=2)

    # W2[pp, i127, ki, j]: k-tile ki = si*2 + io holds slab rows io*128..+128
    W2 = np.zeros((K, P, 4, OUT_FEAT), np.float32)
    for pp, (w, kind) in enumerate(PLANES):
        Wr, Wi = Wf[:, :, w].real.astype(np.float32), Wf[:, :, w].imag.astype(np.float32)
        if w in (0, 8):
            slabs = [Wr]
        elif kind == "re":
            slabs = [Wr, -Wi]
        else:
            slabs = [Wi, Wr]
        for si, S in enumerate(slabs):
            for io_ in range(IO):
                W2[pp, :, si * IO + io_, :] = S[io_ * P:(io_ + 1) * P, :]
    w2 = np.ascontiguousarray(W2, dtype=np.float16)

    in_maps = []
    b1 = B1.astype(np.float16)
    b3 = B3.astype(np.float16)
    for c in range(N_CORES):
        f = field_feat[c].transpose(1, 2, 0)                  # [i, x, bp]
        fx = f.reshape(NIG, 8, K, NUM_PART).transpose(0, 2, 1, 3)
        fx = fx.reshape(NIG // 4, 4, P, NUM_PART).transpose(0, 2, 1, 3)
        fx = np.ascontiguousarray(fx, dtype=np.float16)
        in_maps.append({"fieldx": fx, "b1": b1, "b3": b3, "w2": w2})
    return in_maps


def kernel(field_feat, weights):
    nc = _build()
    in_maps = _prep_inputs(field_feat, weights)
    trace = bool(int(os.environ.get("KERNEL_TRACE", "0")))
    # NRT occasionally reports a transient EXEC_UNIT_UNRECOVERABLE on the
    # first execute of a fresh session; a retry on a new session passes.
    for attempt in range(3):
        try:
            res = run_bass_kernel_spmd(nc, in_maps, list(range(N_CORES)),
                                       trace=trace)
            break
        except Exception:  # noqa: BLE001
            if attempt == 2:
                raise
    if trace:
        kernel.last_exec_time_ns = res.exec_time_ns
        kernel.last_results = res
    # out[jg, j8*16+y, bp] -> [bp, j, y]
    outs = []
    for c in range(N_CORES):
        o = res.results[c]["out"].reshape(NJG, 8, K, NUM_PART)
        outs.append(o.transpose(3, 0, 1, 2).reshape(NUM_PART, OUT_FEAT, K))
    return np.stack(outs).reshape(BATCH, NUM_PART, OUT_FEAT, K).astype(np.float32)
